# revision 18
# baseline (speedup 1.0000x reference)
"""Trainium2 Bass kernel for CSHA attention (ECA channel attention + spatial attention).

Computes, for x [B, C, H, W] = [32, 256, 64, 64]:
    out = x * (1 + ch_w[c] + sp[h, w])
where
    ch_w = sigmoid(conv1d_k5(mean_hw(x)))          (per-channel, ECA)
    sp   = sigmoid(conv2d_7x7([max_c(x); mean_c(x)]) + b)   (per-pixel)

Strategy: pure data parallel over batch across 8 NeuronCores (4 batches per
core), fp16 end-to-end on device (host casts; 2e-2 budget >> fp16 noise),
halving HBM traffic to ~16.8MB/core (~47us roofline).  Per batch:
channel sums via ACT accum; channel max via DVE fold + PE transposes +
DVE reduce; the two pools' convs as PE banded matmuls; sp broadcast into
f16 PSUM via PE transposes of stride-0 views of sigmoid(sp); A = sp+K
materialized from PSUM split across DVE (tensor_scalar add, 2x) and ACT
(Relu with per-partition bias K -- exact since sp+K in (1,3)); final
out = x*A as one in-place fp16 TT multiply per channel-group (2x).
Batch stages are emitted software-pipelined so engines run skewed.
"""

import os
import sys

import numpy as np

sys.path.insert(0, "/opt/trn_rl_repo")

B, C, H, W = 32, 256, 64, 64
HW = H * W            # 4096
N_CORES = 8
BPC = B // N_CORES    # 4 batches per core
H2 = H // 2           # 32 "h2" column blocks (hw = h2*128 + (h%2)*64 + w)

# A-make quarter assignment: (g, quarter) handled by DVE; rest on ACT.
DVE_A_QUARTERS = {(1, 0), (1, 1), (1, 2)}


# ---------------------------------------------------------------------------
# Host-side constant building (tiny, from the conv weights)
# ---------------------------------------------------------------------------

def _build_host_consts(conv1d_w, conv2d_w, conv2d_b):
    """Build matmul-form weight matrices from the conv weights.

    Returns
      w1t  [128, 2, 256] f16 : ECA conv1d as banded matrix, lhsT layout.
                               w1t[cl, kh, co] = W1[co, kh*128+cl] where
                               W1 @ (channel sums) = conv1d(mean) (1/HW folded).
      wd   [128, 10, 128] f16: spatial conv2d as 10 accumulated matmuls in the
                               interleaved layout (partition = (h%2)*64 + w,
                               free = h//2).  wd[:, ch*5+di, :] is the lhsT for
                               channel ch (0=max pool, 1=avg pool; 1/C folded
                               into ch=1) and h2-shift delta = di-2.
      bias [128, 1] f32      : conv2d bias replicated.
    """
    w5 = np.asarray(conv1d_w, np.float32)[0, 0]           # [5]
    W1 = np.zeros((C, C), np.float32)
    for k in range(5):
        co = np.arange(C)
        ci = co + k - 2
        m = (ci >= 0) & (ci < C)
        W1[co[m], ci[m]] = w5[k] / HW
    w1t = W1.T.reshape(2, 128, C).transpose(1, 0, 2).astype(np.float16)  # [128, 2, 256]

    wt = np.asarray(conv2d_w, np.float32)[0].copy()       # [2, 7, 7] (ch, dy, dx)
    wt[1] /= C
    wd = np.zeros((128, 10, 128), np.float32)
    wi = np.arange(64)
    for ch in range(2):
        for di, d in enumerate(range(-2, 3)):
            M = np.zeros((128, 128), np.float32)
            for hp in range(2):
                for hpp in range(2):
                    dy = 2 * d + hp - hpp + 3
                    if not (0 <= dy <= 6):
                        continue
                    # band over w: M[hp*64+w_in, hpp*64+w_out] = wt[ch,dy,w_in-w_out+3]
                    for dx in range(7):
                        w_out = wi
                        w_in = w_out + dx - 3
                        msk = (w_in >= 0) & (w_in < 64)
                        M[hp * 64 + w_in[msk], hpp * 64 + w_out[msk]] = wt[ch, dy, dx]
            wd[:, ch * 5 + di, :] = M
    bias = np.full((128, 1), float(np.asarray(conv2d_b)[0]), np.float32)
    return w1t, wd.astype(np.float16), bias


def _make_in_maps(x, conv1d_w, conv2d_w, conv2d_b):
    x16 = np.asarray(x).astype(np.float16)
    w1t, wd, bias = _build_host_consts(conv1d_w, conv2d_w, conv2d_b)
    shards = x16.reshape(N_CORES, BPC, C, H, W)
    return [
        {"xs": np.ascontiguousarray(shards[i]), "w1t": w1t, "wd": wd, "bias": bias}
        for i in range(N_CORES)
    ]


def _assemble_out(res):
    out = np.concatenate([r["out"] for r in res.results], axis=0)
    return out.reshape(B, C, H, W).astype(np.float32)


# ---------------------------------------------------------------------------
# Device kernel (per core; SPMD over 8 cores)
# ---------------------------------------------------------------------------

def _build_nc():
    import concourse.bass as bass
    import concourse.tile as tile
    from concourse import mybir

    f32 = mybir.dt.float32
    f16 = mybir.dt.float16

    nc = bass.Bass()

    xs_d = nc.dram_tensor("xs", [BPC, C, H, W], f16, kind="ExternalInput")
    w1t_d = nc.dram_tensor("w1t", [128, 2, C], f16, kind="ExternalInput")
    wd_d = nc.dram_tensor("wd", [128, 10, 128], f16, kind="ExternalInput")
    bias_d = nc.dram_tensor("bias", [128, 1], f32, kind="ExternalInput")
    out_d = nc.dram_tensor("out", [BPC, C, H, W], f16, kind="ExternalOutput")

    i128b_d = nc.inline_tensor(np.eye(128, dtype=np.float16), "i128b")
    ones32_d = nc.inline_tensor(np.ones((128, 32), np.float16), "ones32")

    AX = mybir.AxisListType
    ALU = mybir.AluOpType
    ACT = mybir.ActivationFunctionType

    with tile.TileContext(nc) as tc:
        with (
            tc.tile_pool(name="consts", bufs=1) as consts,
            tc.tile_pool(name="xp", bufs=1) as xp,
            tc.tile_pool(name="work", bufs=2) as work,
            tc.tile_pool(name="m1p", bufs=2) as m1p,
            tc.tile_pool(name="junkp", bufs=1) as junkp,
            tc.tile_pool(name="ap", bufs=2) as apool,
            tc.tile_pool(name="psb", bufs=2, space=bass.MemorySpace.PSUM) as psb,
            tc.tile_pool(name="pss", bufs=2, space=bass.MemorySpace.PSUM) as pss,
            tc.tile_pool(name="psS", bufs=2, space=bass.MemorySpace.PSUM) as psS,
        ):
            # ---- constants to SBUF ----
            w1t_t = consts.tile([128, 2, C], f16)
            nc.sync.dma_start(out=w1t_t, in_=w1t_d[:])
            wd_t = consts.tile([128, 10, 128], f16)
            nc.sync.dma_start(out=wd_t, in_=wd_d[:])
            bias_t = consts.tile([128, 1], f32)
            nc.sync.dma_start(out=bias_t, in_=bias_d[:])
            i128b_t = consts.tile([128, 128], f16)
            nc.sync.dma_start(out=i128b_t, in_=i128b_d[:])
            ones32_t = consts.tile([128, 32], f16)
            nc.sync.dma_start(out=ones32_t, in_=ones32_d[:])

            # Dummy matmuls: absorb the const-load DMA waits on PE early so
            # steady-state (ldweights, matmul) pairs stay within the 2-wait
            # hardware budget.
            pd = pss.tile([1, 8], f32, tag="sm")
            for k, (lhs, rhs) in enumerate((
                (w1t_t[:, 0, 0:1], w1t_t[:, 0, 1:2]),
                (wd_t[:, 0, 0:1], wd_t[:, 0, 1:2]),
                (i128b_t[:, 0:1], i128b_t[:, 1:2]),
                (ones32_t[:, 0:1], ones32_t[:, 1:2]),
            )):
                nc.tensor.matmul(pd[:, k : k + 1], lhs, rhs, start=True, stop=True)

            x_tiles = [None] * BPC
            st = {}  # per-batch cross-stage state

            def s_load(b):
                x_t = xp.tile([128, 2, HW], f16, tag=f"x{b}")
                x_tiles[b] = x_t
                # x[b, g*128+cl, hw] -> x_t[cl, g, hw]
                xv = xs_d[b].rearrange("(g c) h w -> c g (h w)", g=2)
                nch = 4 if b == 0 else 1
                csz = HW // nch
                for h in range(nch):
                    sl = slice(h * csz, (h + 1) * csz)
                    for g in range(2):
                        nc.sync.dma_start(out=x_t[:, g, sl], in_=xv[:, g, sl])

            junk0 = None

            def sA(b):
                """Stats over raw x: ACT channel-sum accums, PE column sums,
                DVE fold for channel max."""
                x_t = x_tiles[b]
                yb = work.tile([128, 2], f32, tag=f"y{b}")
                st[b, "yb"] = yb
                m1 = m1p.tile([128, HW], f16, tag="m1")
                st[b, "m1"] = m1
                if b == 0:
                    yb4 = work.tile([128, 2, 2], f32, tag="y4")
                    for h in range(2):
                        sl = slice(h * 2048, (h + 1) * 2048)
                        for g in range(2):
                            nc.scalar.activation(
                                out=junk0[:, sl],
                                in_=x_t[:, g, sl],
                                func=ACT.Copy,
                                accum_out=yb4[:, g, h : h + 1],
                            )
                        nc.vector.tensor_max(
                            m1[:, sl], x_t[:, 0, sl], x_t[:, 1, sl]
                        )
                    nc.vector.tensor_add(yb, yb4[:, :, 0], yb4[:, :, 1])
                else:
                    for g in range(2):
                        nc.scalar.activation(
                            out=junk0,
                            in_=x_t[:, g, :],
                            func=ACT.Copy,
                            accum_out=yb[:, g : g + 1],
                        )
                    nc.vector.tensor_max(m1, x_t[:, 0, :], x_t[:, 1, :])

                for i in range(2):
                    psc = pss.tile([128, 512], f32, tag="cs")
                    st[b, "psc", i] = psc
                    for q in range(4):
                        j = i * 4 + q
                        o = psc[32 * q : 32 * q + 32, :]
                        for g in range(2):
                            nc.tensor.matmul(
                                o,
                                ones32_t,
                                x_t[:, g, j * 512 : (j + 1) * 512],
                                start=(g == 0),
                                stop=(g == 1),
                                tile_position=(0, 32 * q),
                            )

            def sB(b):
                """Pool maps: avg map via ACT copy + PE transposes; max map
                via PE transposes of m1 + DVE reduces."""
                m1 = st[b, "m1"]
                cs_full = work.tile([128, 2, 512], f16, tag="cs_sb", bufs=3)
                nc.vector.tensor_copy(cs_full[:, 0, :], st[b, "psc", 0])
                nc.scalar.activation(
                    out=cs_full[:, 1, :], in_=st[b, "psc", 1], func=ACT.Copy
                )

                ap_map = work.tile([128, 36], f16, tag="apm", bufs=3)
                st[b, "ap_map"] = ap_map
                nc.vector.memset(
                    ap_map.rearrange("p (a b) -> p a b", a=18)[:, 0:18:17, :], 0.0
                )
                pfull = psb.tile([128, 2, 4, 128], f16, tag="big")
                for i in range(2):
                    for s1 in range(4):
                        nc.tensor.transpose(
                            out=pfull[:, i, s1, :],
                            in_=cs_full[:, i, s1 * 128 : (s1 + 1) * 128],
                            identity=i128b_t,
                        )
                # ap_map col h2 = (i*4+q)*4 + s1  <-  pfull[:, i, s1, 32*q]
                nc.vector.tensor_copy(
                    out=ap_map[:, 2:34].rearrange("p (i q s) -> p i s q", i=2, q=4),
                    in_=pfull[:, :, :, 0:97:32],
                )

                mp_map = work.tile([128, 36], f16, tag="mpm", bufs=3)
                st[b, "mp_map"] = mp_map
                nc.vector.memset(
                    mp_map.rearrange("p (a b) -> p a b", a=18)[:, 0:18:17, :], 0.0
                )
                for t in range(4):
                    pmt = psb.tile([128, 8, 128], f16, tag="big")
                    for k in range(8):
                        nc.tensor.transpose(
                            out=pmt[:, k, :],
                            in_=m1[:, (t * 8 + k) * 128 : (t * 8 + k + 1) * 128],
                            identity=i128b_t,
                        )
                    nc.vector.reduce_max(
                        out=mp_map[:, 2 + t * 8 : 2 + t * 8 + 8], in_=pmt, axis=AX.X
                    )

            def sC(b):
                """Tiny convs: spatial 7x7 + sigmoid; ECA conv1d + sigmoid + 1."""
                psp = pss.tile([128, 32], f32, tag="sm")
                for ch in range(2):
                    mm = st[b, "mp_map"] if ch == 0 else st[b, "ap_map"]
                    for di in range(5):
                        nc.tensor.matmul(
                            psp,
                            wd_t[:, ch * 5 + di, :],
                            mm[:, di : di + 32],
                            start=(ch == 0 and di == 0),
                            stop=(ch == 1 and di == 4),
                        )
                sph = work.tile([128, 32], f16, tag="sph", bufs=3)
                st[b, "sph"] = sph
                nc.scalar.activation(
                    out=sph, in_=psp, func=ACT.Sigmoid, bias=bias_t[:, 0:1]
                )

                ybh = work.tile([128, 2], f16, tag="ybh", bufs=3)
                nc.gpsimd.tensor_copy(ybh, st[b, "yb"])
                pchw = pss.tile([128, 2], f32, tag="sm")
                for hp in range(2):
                    for kh in range(2):
                        nc.tensor.matmul(
                            pchw[:, hp : hp + 1],
                            w1t_t[:, kh, hp * 128 : (hp + 1) * 128],
                            ybh[:, kh : kh + 1],
                            start=(kh == 0),
                            stop=(kh == 1),
                        )
                chw1 = work.tile([128, 2], f32, tag="chw", bufs=3)
                nc.scalar.activation(out=chw1, in_=pchw, func=ACT.Sigmoid)
                nc.vector.tensor_scalar_add(chw1, chw1, 1.0)
                st[b, "K"] = chw1

            def sD(b, chunks=2):
                """sp broadcast to f16 PSUM (PE), A = sp + K[g] (DVE/ACT split),
                apply out = x*A in place (DVE TT), store per channel-group."""
                x_t = x_tiles[b]
                sph = st[b, "sph"]
                K = st[b, "K"]
                A0 = apool.tile([128, HW], f16, tag="A0")
                A1 = apool.tile([128, HW], f16, tag="A1")
                A = {0: A0, 1: A1}
                for quarter in range(4):
                    ps = psS.tile([128, 1024], f16, tag="S")
                    for j in range(8):
                        h2 = quarter * 8 + j
                        nc.tensor.transpose(
                            out=ps[:, j * 128 : (j + 1) * 128],
                            in_=sph[:, h2 : h2 + 1].broadcast_to([128, 128]),
                            identity=i128b_t,
                        )
                    for g in range(2):
                        dst = A[g][:, quarter * 1024 : (quarter + 1) * 1024]
                        if (g, quarter) in DVE_A_QUARTERS:
                            nc.vector.tensor_scalar(
                                out=dst, in0=ps, scalar1=K[:, g : g + 1],
                                scalar2=None, op0=ALU.add,
                            )
                        else:
                            # Relu(sp + K) == sp + K exactly: sp+K in (1, 3)
                            nc.scalar.activation(
                                out=dst, in_=ps, func=ACT.Relu,
                                bias=K[:, g : g + 1],
                            )
                ov = out_d[b].rearrange("(g c) h w -> c g (h w)", g=2)
                csz = HW // chunks
                for h in range(chunks):
                    for g in range(2):
                        sl = slice(h * csz, (h + 1) * csz)
                        xsl = x_t[:, g, sl]
                        nc.vector.tensor_mul(xsl, A[g][:, sl], xsl)
                        nc.sync.dma_start(out=ov[:, g, sl], in_=xsl)

            # ---- emission: software-pipelined across batches ----
            for b in range(BPC):
                s_load(b)

            junk0 = junkp.tile([128, HW], f16, tag="junk")
            jb = work.tile([128, 1], f32, tag="junk0")
            nc.scalar.activation(out=jb, in_=bias_t, func=ACT.Copy)

            sA(0)
            sA(1)
            sB(0)
            sC(0)
            sA(2)
            sB(1)
            sD(0)
            sC(1)
            sA(3)
            sB(2)
            sD(1)
            sC(2)
            sB(3)
            sC(3)
            sD(2)
            sD(3, chunks=4)

    _split_excess_waits(nc, mybir)
    return nc


def _split_excess_waits(nc, mybir):
    """Walrus limits sync-wait commands per instruction (1 for compute
    engine instructions, ~2 for DMA).  Tile can emit more when an
    instruction depends on several engines.  Move the excess waits onto an
    inserted same-engine NoOp immediately before the instruction — engine
    program order makes this equivalent."""
    SKIP = (mybir.InstNoOp, mybir.InstAllEngineBarrier)
    for fn in nc.m.functions:
        for blk in fn.blocks:
            new = []
            for inst in blk.instructions:
                si = inst.sync_info
                if si is not None and si.on_wait and not isinstance(inst, SKIP):
                    waits = list(si.on_wait)
                    if len(waits) > 1:
                        moved, keep = waits[:-1], waits[-1:]
                        for k, w in enumerate(moved):
                            nop = mybir.InstNoOp(
                                name=f"{inst.name}-wsplit{k}",
                                engine=inst.engine,
                                sync_info=mybir.SyncInfo(on_wait=[w], on_update=[]),
                                bass_nofuse=True,
                            )
                            new.append(nop)
                        si.on_wait = keep
                new.append(inst)
            blk.instructions[:] = new


# ---------------------------------------------------------------------------
# Entry point
# ---------------------------------------------------------------------------

def kernel(x, conv1d_w, conv2d_w, conv2d_b):
    from concourse.bass_utils import run_bass_kernel_spmd

    nc = _build_nc()
    in_maps = _make_in_maps(x, conv1d_w, conv2d_w, conv2d_b)
    res = run_bass_kernel_spmd(nc, in_maps, core_ids=list(range(N_CORES)))
    return _assemble_out(res)
